# revision 14
# baseline (speedup 1.0000x reference)
"""AdaptiveFilterBank Trainium2 kernel (8 NeuronCores, data-parallel over batch).

Math: reference = conv1d(x, filters) then per-sample softmax-weighted sum over
the 8 filter channels. The weighted sum commutes with the (linear) conv, so
each sample needs ONE length-31 conv with a combined per-sample filter
    kb[b] = softmax(MLP(features[b])) @ filter_params      (tiny, host-side)

Device formulation (overlapped interleave, P=98): per sample lay x out as
    X[q, c] = x[c*98 + q - 15]      (zero-padded), [128, 1338] in SBUF
so each SBUF column holds a 128-wide window covering the 98 outputs of that
column plus the +-15 conv halo. Then the whole 'same' cross-correlation is ONE
matmul per output tile:
    Y[m, c] = sum_q T[q, m] X[q, c],   T[q, m] = kb[q - m]  (0 <= q-m <= 30)
with Y[m, c] = y[c*98 + m], m in [0, 98).

All PE traffic is bf16 (PE measured pinned at 1.2 GHz, 1 col/cycle; fp32r was
~2x slower end-to-end); accumulation is fp32 in PSUM; HBM traffic is bf16 both
ways. End-to-end error vs fp32 reference ~4e-3 (gate 2e-2).

Sharding: batch 64 -> 8 samples per core; filter/MLP params host-computed.
"""

import numpy as np

B = 64
L = 131072
N_CORES = 8
BPC = B // N_CORES          # samples per core
KLEN = 31
PAD = 15
P = 98                      # outputs per interleave column (128 - 30 halo)
NCOLS = 1338                # ceil(L / P) input/output columns per sample
NSPLIT = (512, 512, 314)    # matmul N tiling of the 1338 columns
_CACHE = {}


def _build_graph():
    """Raw Bacc graph with hand-rolled semaphores (Tile's fixed epilogue —
    kernel-tail drain + EVSEM butterfly — measured ~9 us, so we skip Tile)."""
    from concourse import bacc, mybir

    dt = mybir.dt
    nc = bacc.Bacc("TRN2", target_bir_lowering=False, debug=False,
                   num_devices=N_CORES)

    x_ext = nc.dram_tensor("xt", [128, BPC * NCOLS], dt.bfloat16,
                           kind="ExternalInput").ap()
    t_ext = nc.dram_tensor("tw", [128, BPC * P], dt.bfloat16,
                           kind="ExternalInput").ap()
    out_ext = nc.dram_tensor("out", [P, BPC * NCOLS], dt.bfloat16,
                             kind="ExternalOutput").ap()

    NOT = 4                 # output staging slots
    # input DMA chunks (in samples): small first chunk so PE starts early
    chunk_sizes = [1, 1, 2, 2, 2]
    chunk_of = []           # sample -> chunk index
    for ci, cs in enumerate(chunk_sizes):
        chunk_of += [ci] * cs
    c0s = [sum(NSPLIT[:h]) for h in range(len(NSPLIT))]
    # engine that copies tile (b, h): ACT for the middle tile of every sample
    # plus the short tile of the last two samples (load balance)
    def is_act(b, h):
        return h == 1 or (h == 2 and b >= BPC - 2)

    def copies_done_before(k):
        """(#DVE, #ACT) copies among global tiles 0..k-1."""
        nv = sum(1 for j in range(k)
                 if not is_act(j // len(NSPLIT), j % len(NSPLIT)))
        ns = k - nv
        return nv, ns

    from contextlib import ExitStack
    stack = ExitStack()
    with (
        nc.sbuf_tensor("xt_sb", [128, BPC * NCOLS], dt.bfloat16) as xt_sb,
        nc.sbuf_tensor("tw_sb", [128, BPC * P], dt.bfloat16) as tw_sb,
        nc.sbuf_tensor("ot_sb", [P, NOT * NCOLS], dt.bfloat16) as ot_sb,
        nc.psum_tensor("ps", [P, 8 * 512], dt.float32) as ps,
        nc.semaphore("s_tw") as s_tw,
        nc.semaphore("s_mm") as s_mm,
        nc.semaphore("s_cv") as s_cv,
        nc.semaphore("s_cs") as s_cs,
        nc.semaphore("s_done") as s_done,
        stack,
    ):
        # one sem per input chunk / per output staging slot: a sem shared by
        # several DMAs can't prove WHICH one finished (partial interleavings)
        s_ch = [stack.enter_context(nc.semaphore(f"s_ch{i}"))
                for i in range(len(chunk_sizes))]
        s_sl = [stack.enter_context(nc.semaphore(f"s_sl{i}"))
                for i in range(NOT)]
        block_cm = nc.Block(no_gpsimd_drain=True)
        block = block_cm.__enter__()

        def wait_out_done(eng, s):
            """Wait until sample s's output DMA completed (slot sem, exact)."""
            eng.wait_ge(s_sl[s % NOT], 16 * (s // NOT + 1))

        def emit_out_dma(eng, b):
            nv, ns = copies_done_before(len(NSPLIT) * (b + 1))
            eng.wait_ge(s_cv, nv)
            eng.wait_ge(s_cs, ns)
            so = (b % NOT) * NCOLS
            eng.dma_start(out=out_ext[:, b * NCOLS:(b + 1) * NCOLS],
                          in_=ot_sb[:, so:so + NCOLS]).then_inc(s_sl[b % NOT], 16)

        @block.sync
        def _(sync):
            sync.dma_start(out=tw_sb[:], in_=t_ext[:]).then_inc(s_tw, 16)
            lo = 0
            for ci, cs in enumerate(chunk_sizes):
                hi = lo + cs * NCOLS
                sync.dma_start(out=xt_sb[:, lo:hi],
                               in_=x_ext[:, lo:hi]).then_inc(s_ch[ci], 16)
                lo = hi
            for b in range(0, BPC, 2):       # even samples out on sync ring
                emit_out_dma(sync, b)
            for i in range(NOT):
                sync.wait_ge(s_sl[i], 32)    # both outs per slot done
            sync.nop().then_inc(s_done, 1)

        @block.tensor
        def _(tensor):
            for b in range(BPC):
                if b == 0:
                    tensor.wait_ge(s_tw, 16)
                tensor.wait_ge(s_ch[chunk_of[b]], 16)
                for h, n in enumerate(NSPLIT):
                    k = len(NSPLIT) * b + h
                    if k >= 8:
                        nv, ns = copies_done_before(k - 7)
                        tensor.wait_ge(s_cv, nv)
                        tensor.wait_ge(s_cs, ns)
                    bank = (k % 8) * 512
                    c0 = c0s[h]
                    tensor.matmul(
                        ps[:, bank:bank + n],
                        tw_sb[:, b * P:(b + 1) * P],
                        xt_sb[:, b * NCOLS + c0:b * NCOLS + c0 + n],
                        start=True, stop=True).then_inc(s_mm, 1)

        @block.vector
        def _(vector):
            for b in range(BPC):
                so = (b % NOT) * NCOLS
                first = True
                for h, n in enumerate(NSPLIT):
                    if is_act(b, h):
                        continue
                    k = len(NSPLIT) * b + h
                    vector.wait_ge(s_mm, k + 1)
                    if b >= NOT and first:
                        wait_out_done(vector, b - NOT)
                    first = False
                    bank = (k % 8) * 512
                    c0 = c0s[h]
                    vector.tensor_copy(ot_sb[:, so + c0:so + c0 + n],
                                       ps[:, bank:bank + n]).then_inc(s_cv, 1)

        @block.scalar
        def _(scalar):
            for b in range(BPC):
                so = (b % NOT) * NCOLS
                first = True
                for h, n in enumerate(NSPLIT):
                    if not is_act(b, h):
                        continue
                    k = len(NSPLIT) * b + h
                    scalar.wait_ge(s_mm, k + 1)
                    if b >= NOT and first:
                        wait_out_done(scalar, b - NOT)
                    first = False
                    bank = (k % 8) * 512
                    c0 = c0s[h]
                    scalar.copy(ot_sb[:, so + c0:so + c0 + n],
                                ps[:, bank:bank + n]).then_inc(s_cs, 1)
                if b % 2 == 1:               # odd samples out on scalar ring
                    emit_out_dma(scalar, b)

        @block.gpsimd
        def _(gpsimd):
            gpsimd.wait_ge(s_done, 1)

        # block exit emits drain + all-engine barrier; then reset the kernel
        # sems to 0 so the NEFF can re-execute
        block_cm.__exit__(None, None, None)
        nums = sorted(s.num for s in
                      [s_tw, s_mm, s_cv, s_cs, s_done] + s_ch + s_sl)
        nc.gpsimd.dma_reset(range(nums[0], nums[-1] + 1))
        nc.gpsimd.sem_clear(range(nums[0], nums[-1] + 1))

    nc.compile()
    return nc


def _get_graph():
    if "nc" not in _CACHE:
        _CACHE["nc"] = _build_graph()
    return _CACHE["nc"]


def _host_prep(x, features, filter_params, W1, b1, W2, b2):
    """Selector MLP + combined filters + layout prep. All tiny or memory-bound."""
    import ml_dtypes
    from numpy.lib.stride_tricks import sliding_window_view
    bf16 = ml_dtypes.bfloat16

    x = np.ascontiguousarray(x, dtype=np.float32)
    # selector MLP (torch Linear convention)
    h = np.maximum(features @ W1.T + b1, 0.0)
    logits = h @ W2.T + b2
    e = np.exp(logits - logits.max(axis=-1, keepdims=True))
    w = e / e.sum(axis=-1, keepdims=True)                      # (B, 8)
    kb = (w @ filter_params[:, 0, :]).astype(np.float32)       # (B, 31)

    # overlapped interleave: X[b, q, c] = x[b, c*98 + q - 15]
    span = (NCOLS - 1) * P + 128
    xp = np.zeros((B, span), dtype=np.float32)
    xp[:, PAD:PAD + L] = x
    win = sliding_window_view(xp, 128, axis=1)                 # (B, span-127, 128)
    xt = win[:, ::P][:, :NCOLS].transpose(0, 2, 1)             # (B, 128, 1338)

    # banded Toeplitz weight: T[q, m] = kb[q - m], 0 <= q-m <= 30
    q = np.arange(128)[:, None]
    m = np.arange(P)[None, :]
    t_i = q - m
    mask = (t_i >= 0) & (t_i <= 30)
    tw = np.zeros((B, 128, P), dtype=np.float32)
    tw[:, mask] = kb[:, t_i[mask]]

    def pack(a):  # (B, Pdim, C) -> per-core (Pdim, BPC*C) bf16
        Pd, C = a.shape[1], a.shape[2]
        return [np.ascontiguousarray(
                    a[i * BPC:(i + 1) * BPC].transpose(1, 0, 2).reshape(Pd, BPC * C)
                ).astype(bf16) for i in range(N_CORES)]

    return pack(xt), pack(tw)


def _run(inputs, trace=False, trace_cores=None):
    """Shard, execute on 8 NeuronCores, gather. Returns (y, exec_time_ns)."""
    from concourse.bass_utils import run_bass_kernel_spmd

    xts, tws = _host_prep(**inputs)
    nc = _get_graph()
    in_maps = [{"xt": xts[i], "tw": tws[i]} for i in range(N_CORES)]
    res = run_bass_kernel_spmd(nc, in_maps, core_ids=list(range(N_CORES)),
                               trace=trace, trace_cores=trace_cores)
    # gather: per-core out [P, BPC*NCOLS]; sample block.T.flatten()[:L] -> y[b]
    y = np.empty((B, L), dtype=np.float32)
    for i in range(N_CORES):
        yc = np.asarray(res.results[i]["out"]).astype(np.float32)
        yc = yc.reshape(P, BPC, NCOLS).transpose(1, 2, 0)      # (BPC, NCOLS, P)
        y[i * BPC:(i + 1) * BPC] = yc.reshape(BPC, NCOLS * P)[:, :L]
    return y, res.exec_time_ns


def kernel(x, features, filter_params, W1, b1, W2, b2):
    y, _ = _run(dict(x=x, features=features, filter_params=filter_params,
                     W1=W1, b1=b1, W2=W2, b2=b2))
    return y


# revision 18
# speedup vs baseline: 1.1430x; 1.1430x over previous
"""AdaptiveFilterBank Trainium2 kernel (8 NeuronCores, data-parallel over batch).

Math: reference = conv1d(x, filters) then per-sample softmax-weighted sum over
the 8 filter channels. The weighted sum commutes with the (linear) conv, so
each sample needs ONE length-31 conv with a combined per-sample filter
    kb[b] = softmax(MLP(features[b])) @ filter_params      (tiny, host-side)

Device formulation (overlapped interleave, P=98): per sample lay x out as
    X[q, c] = x[c*98 + q - 15]      (zero-padded), [128, 1338] in SBUF
so each SBUF column holds a 128-wide window covering the 98 outputs of that
column plus the +-15 conv halo. Then the whole 'same' cross-correlation is ONE
matmul per output tile:
    Y[m, c] = sum_q T[q, m] X[q, c],   T[q, m] = kb[q - m]  (0 <= q-m <= 30)
with Y[m, c] = y[c*98 + m], m in [0, 98).

All PE traffic is bf16 (PE measured pinned at 1.2 GHz, 1 col/cycle; fp32r was
~2x slower end-to-end); accumulation is fp32 in PSUM; HBM traffic is bf16 both
ways. End-to-end error vs fp32 reference ~4e-3 (gate 2e-2).

Sharding: batch 64 -> 8 samples per core; filter/MLP params host-computed.
"""

import numpy as np

B = 64
L = 131072
N_CORES = 8
BPC = B // N_CORES          # samples per core
KLEN = 31
PAD = 15
P = 98                      # outputs per interleave column (128 - 30 halo)
NCOLS = 1338                # ceil(L / P) input/output columns per sample
NSPLIT = (512, 512, 314)    # matmul N tiling of the 1338 columns
_CACHE = {}


def _build_graph():
    """Raw Bacc graph with hand-rolled semaphores (Tile's fixed epilogue —
    kernel-tail drain + EVSEM butterfly — measured ~9 us, so we skip Tile)."""
    from concourse import bacc, mybir

    dt = mybir.dt
    nc = bacc.Bacc("TRN2", target_bir_lowering=False, debug=False,
                   num_devices=N_CORES)

    x_ext = nc.dram_tensor("xt", [128, BPC * NCOLS], dt.bfloat16,
                           kind="ExternalInput").ap()
    t_ext = nc.dram_tensor("tw", [128, BPC * P], dt.bfloat16,
                           kind="ExternalInput").ap()
    out_ext = nc.dram_tensor("out", [P, BPC * NCOLS], dt.bfloat16,
                             kind="ExternalOutput").ap()

    NOT = 4                 # output staging slots
    # input DMA chunks (in samples): small first chunk so PE starts early
    chunk_sizes = [1, 1, 2, 4]
    chunk_of = []           # sample -> chunk index
    for ci, cs in enumerate(chunk_sizes):
        chunk_of += [ci] * cs
    c0s = [sum(NSPLIT[:h]) for h in range(len(NSPLIT))]
    # engine that copies tile (b, h): ACT for the middle tile of every sample
    # plus the short tile of the last two samples (load balance)
    def is_act(b, h):
        return h == 1 or (h == 2 and b >= BPC - 2)

    def copies_done_before(k):
        """(#DVE, #ACT) copies among global tiles 0..k-1."""
        nv = sum(1 for j in range(k)
                 if not is_act(j // len(NSPLIT), j % len(NSPLIT)))
        ns = k - nv
        return nv, ns

    from contextlib import ExitStack
    stack = ExitStack()
    with (
        nc.sbuf_tensor("xt_sb", [128, BPC * NCOLS], dt.bfloat16) as xt_sb,
        nc.sbuf_tensor("tw_sb", [128, BPC * P], dt.bfloat16) as tw_sb,
        nc.sbuf_tensor("ot_sb", [P, NOT * NCOLS], dt.bfloat16) as ot_sb,
        nc.psum_tensor("ps", [P, 8 * 512], dt.float32) as ps,
        nc.semaphore("s_tw") as s_tw,
        nc.semaphore("s_mm") as s_mm,
        nc.semaphore("s_cv") as s_cv,
        nc.semaphore("s_cs") as s_cs,
        nc.semaphore("s_done") as s_done,
        stack,
    ):
        # one sem per input chunk / per output staging slot: a sem shared by
        # several DMAs can't prove WHICH one finished (partial interleavings)
        s_ch = [stack.enter_context(nc.semaphore(f"s_ch{i}"))
                for i in range(len(chunk_sizes))]
        s_pr = [stack.enter_context(nc.semaphore(f"s_pr{i}"))
                for i in range(NOT // 2)]    # one per staging slot PAIR
        block_cm = nc.Block(no_gpsimd_drain=True)
        block = block_cm.__enter__()

        def wait_out_done(eng, s):
            """Wait until sample s's (paired) output DMA completed."""
            eng.wait_ge(s_pr[(s % NOT) // 2], 16 * (s // NOT + 1))

        @block.sync
        def _(sync):
            sync.dma_start(out=tw_sb[:], in_=t_ext[:]).then_inc(s_tw, 16)
            lo = 0
            for ci, cs in enumerate(chunk_sizes):
                hi = lo + cs * NCOLS
                sync.dma_start(out=xt_sb[:, lo:hi],
                               in_=x_ext[:, lo:hi]).then_inc(s_ch[ci], 16)
                lo = hi
            for b in range(0, BPC, 2):       # paired outputs, all on sync ring
                nv, ns = copies_done_before(len(NSPLIT) * (b + 2))
                sync.wait_ge(s_cv, nv)
                sync.wait_ge(s_cs, ns)
                so = (b % NOT) * NCOLS
                sync.dma_start(
                    out=out_ext[:, b * NCOLS:(b + 2) * NCOLS],
                    in_=ot_sb[:, so:so + 2 * NCOLS],
                ).then_inc(s_pr[(b % NOT) // 2], 16)
            for i in range(NOT // 2):
                sync.wait_ge(s_pr[i], 32)    # both pair-outs per slot-pair done
            sync.nop().then_inc(s_done, 1)

        @block.tensor
        def _(tensor):
            for b in range(BPC):
                if b == 0:
                    tensor.wait_ge(s_tw, 16)
                tensor.wait_ge(s_ch[chunk_of[b]], 16)
                for h, n in enumerate(NSPLIT):
                    k = len(NSPLIT) * b + h
                    if k >= 8:
                        nv, ns = copies_done_before(k - 7)
                        tensor.wait_ge(s_cv, nv)
                        tensor.wait_ge(s_cs, ns)
                    bank = (k % 8) * 512
                    c0 = c0s[h]
                    tensor.matmul(
                        ps[:, bank:bank + n],
                        tw_sb[:, b * P:(b + 1) * P],
                        xt_sb[:, b * NCOLS + c0:b * NCOLS + c0 + n],
                        start=True, stop=True).then_inc(s_mm, 1)

        @block.vector
        def _(vector):
            for b in range(BPC):
                so = (b % NOT) * NCOLS
                first = True
                for h, n in enumerate(NSPLIT):
                    if is_act(b, h):
                        continue
                    k = len(NSPLIT) * b + h
                    vector.wait_ge(s_mm, k + 1)
                    if b >= NOT and first:
                        wait_out_done(vector, b - NOT)
                    first = False
                    bank = (k % 8) * 512
                    c0 = c0s[h]
                    vector.tensor_copy(ot_sb[:, so + c0:so + c0 + n],
                                       ps[:, bank:bank + n]).then_inc(s_cv, 1)

        @block.scalar
        def _(scalar):
            for b in range(BPC):
                so = (b % NOT) * NCOLS
                first = True
                for h, n in enumerate(NSPLIT):
                    if not is_act(b, h):
                        continue
                    k = len(NSPLIT) * b + h
                    scalar.wait_ge(s_mm, k + 1)
                    if b >= NOT and first:
                        wait_out_done(scalar, b - NOT)
                    first = False
                    bank = (k % 8) * 512
                    c0 = c0s[h]
                    scalar.copy(ot_sb[:, so + c0:so + c0 + n],
                                ps[:, bank:bank + n]).then_inc(s_cs, 1)
        @block.gpsimd
        def _(gpsimd):
            gpsimd.wait_ge(s_done, 1)

        # block exit emits drain + all-engine barrier; then reset the kernel
        # sems to 0 so the NEFF can re-execute
        block_cm.__exit__(None, None, None)
        nums = sorted(s.num for s in
                      [s_tw, s_mm, s_cv, s_cs, s_done] + s_ch + s_pr)
        nc.gpsimd.dma_reset(range(nums[0], nums[-1] + 1))
        nc.gpsimd.sem_clear(range(nums[0], nums[-1] + 1))

    nc.compile()
    return nc


def _get_graph():
    if "nc" not in _CACHE:
        _CACHE["nc"] = _build_graph()
    return _CACHE["nc"]


def _host_prep(x, features, filter_params, W1, b1, W2, b2):
    """Selector MLP + combined filters + layout prep. All tiny or memory-bound."""
    import ml_dtypes
    from numpy.lib.stride_tricks import sliding_window_view
    bf16 = ml_dtypes.bfloat16

    x = np.ascontiguousarray(x, dtype=np.float32)
    # selector MLP (torch Linear convention)
    h = np.maximum(features @ W1.T + b1, 0.0)
    logits = h @ W2.T + b2
    e = np.exp(logits - logits.max(axis=-1, keepdims=True))
    w = e / e.sum(axis=-1, keepdims=True)                      # (B, 8)
    kb = (w @ filter_params[:, 0, :]).astype(np.float32)       # (B, 31)

    # overlapped interleave: X[b, q, c] = x[b, c*98 + q - 15]
    span = (NCOLS - 1) * P + 128
    xp = np.zeros((B, span), dtype=np.float32)
    xp[:, PAD:PAD + L] = x
    win = sliding_window_view(xp, 128, axis=1)                 # (B, span-127, 128)
    xt = win[:, ::P][:, :NCOLS].transpose(0, 2, 1)             # (B, 128, 1338)

    # banded Toeplitz weight: T[q, m] = kb[q - m], 0 <= q-m <= 30
    q = np.arange(128)[:, None]
    m = np.arange(P)[None, :]
    t_i = q - m
    mask = (t_i >= 0) & (t_i <= 30)
    tw = np.zeros((B, 128, P), dtype=np.float32)
    tw[:, mask] = kb[:, t_i[mask]]

    def pack(a):  # (B, Pdim, C) -> per-core (Pdim, BPC*C) bf16
        Pd, C = a.shape[1], a.shape[2]
        return [np.ascontiguousarray(
                    a[i * BPC:(i + 1) * BPC].transpose(1, 0, 2).reshape(Pd, BPC * C)
                ).astype(bf16) for i in range(N_CORES)]

    return pack(xt), pack(tw)


def _run(inputs, trace=False, trace_cores=None):
    """Shard, execute on 8 NeuronCores, gather. Returns (y, exec_time_ns)."""
    from concourse.bass_utils import run_bass_kernel_spmd

    xts, tws = _host_prep(**inputs)
    nc = _get_graph()
    in_maps = [{"xt": xts[i], "tw": tws[i]} for i in range(N_CORES)]
    res = run_bass_kernel_spmd(nc, in_maps, core_ids=list(range(N_CORES)),
                               trace=trace, trace_cores=trace_cores)
    # gather: per-core out [P, BPC*NCOLS]; sample block.T.flatten()[:L] -> y[b]
    y = np.empty((B, L), dtype=np.float32)
    for i in range(N_CORES):
        yc = np.asarray(res.results[i]["out"]).astype(np.float32)
        yc = yc.reshape(P, BPC, NCOLS).transpose(1, 2, 0)      # (BPC, NCOLS, P)
        y[i * BPC:(i + 1) * BPC] = yc.reshape(BPC, NCOLS * P)[:, :L]
    return y, res.exec_time_ns


def kernel(x, features, filter_params, W1, b1, W2, b2):
    y, _ = _run(dict(x=x, features=features, filter_params=filter_params,
                     W1=W1, b1=b1, W2=W2, b2=b2))
    return y
